# revision 13
# baseline (speedup 1.0000x reference)
"""Multi-head self-attention (B=2, T=2048, C=1024, H=16) on 8 NeuronCores.

Sharding: core c -> (batch b = c//4, head-group g = c%4); each core computes
4 heads' attention for one batch plus its slice of the QKV/out projections.
Per-core partial outputs (over head groups) are summed on the host.

Device-side layout is fully transposed (feature dim on partitions):
  xt [C, T] -> QT/KT [256, T] (j on partitions), V natural [T, 256],
  ST = K Qt (scores transposed, tk on partitions).
The stationary PV operand is V extended with 64 columns of ones, so the
yext accumulator's rows 64..127 all hold the softmax denominator — a free
hardware broadcast that lets normalization run entirely on VectorE
(reciprocal + multiply) without touching the PE or ScalarE.
Exp runs on ScalarE over 2-PSUM-bank tiles to amortize the ~352-cycle
per-instruction overhead. Matmul operands are fp16 (10-bit mantissa,
~6e-4 end-to-end rel err) with fp32 PSUM accumulation. bq/bk (and the
1/sqrt(hd) scale) are folded in on the host; bv/bo commute through softmax
(rows sum to 1) and are added on the host at the end.
"""

import numpy as np

import concourse.bacc as bacc
import concourse.mybir as mybir
import concourse.tile as tile
from concourse.bass_utils import run_bass_kernel_spmd

B, T, C, H = 2, 2048, 1024, 16
HD = C // H  # 64
NCORES = 8
GROUPS = 4  # head groups (one per core within a batch)
HPG = H // GROUPS  # heads per group = 4
JW = HPG * HD  # per-core projection slice width = 256

F32 = mybir.dt.float32
MMDT = mybir.dt.float16
NPDT = np.float16

_CACHED_NC = None


def _outproj(nc, psA, outp, yt_sb, wo_sb, out, qb):
    for tt in range(qb * 4, (qb + 1) * 4):
        po = psA.tile([128, 1024], F32, tag="mm", name="po")
        for mb in range(2):
            for jt in range(2):
                nc.tensor.matmul(
                    po[:, mb * 512 : (mb + 1) * 512],
                    yt_sb[:, jt, tt * 128 : (tt + 1) * 128],
                    wo_sb[:, jt, mb * 512 : (mb + 1) * 512],
                    start=(jt == 0),
                    stop=(jt == 1),
                )
        ob = outp.tile([128, 1024], F32, tag="ob", name="ob")
        nc.vector.tensor_copy(out=ob[:], in_=po[:])
        nc.sync.dma_start(out=out[tt * 128 : (tt + 1) * 128, :], in_=ob[:])


def _build():
    nc = bacc.Bacc("TRN2", target_bir_lowering=False, num_devices=NCORES)

    xt = nc.dram_tensor("xt", [C, T], MMDT, kind="ExternalInput")
    wq = nc.dram_tensor("wq", [C, JW], MMDT, kind="ExternalInput")
    wk = nc.dram_tensor("wk", [C, JW], MMDT, kind="ExternalInput")
    wv = nc.dram_tensor("wv", [C, JW], MMDT, kind="ExternalInput")
    wo = nc.dram_tensor("wo", [JW, C], MMDT, kind="ExternalInput")
    bq = nc.dram_tensor("bq", [JW], F32, kind="ExternalInput")
    bk = nc.dram_tensor("bk", [JW], F32, kind="ExternalInput")
    out = nc.dram_tensor("out", [T, C], F32, kind="ExternalOutput")

    xt_ap = xt[:, :].rearrange("(cc p) t -> p cc t", p=128)  # [128, 8, T]
    wq_ap = wq[:, :].rearrange("(cc p) j -> p cc j", p=128)  # [128, 8, 256]
    wk_ap = wk[:, :].rearrange("(cc p) j -> p cc j", p=128)
    wv_ap = wv[:, :].rearrange("(cc p) j -> p cc j", p=128)
    wo_ap = wo[:, :].rearrange("(jt p) m -> p jt m", p=128)  # [128, 2, C]
    bq_ap = bq[:].rearrange("(jt p) -> p jt", p=128)  # [128, 2]
    bk_ap = bk[:].rearrange("(jt p) -> p jt", p=128)

    with tile.TileContext(nc) as tc:
        with (
            tc.tile_pool(name="big", bufs=1) as big,
            tc.tile_pool(name="work", bufs=6) as work,
            tc.tile_pool(name="nrm", bufs=4) as nrm,
            tc.tile_pool(name="outp", bufs=3) as outp,
            tc.tile_pool(name="psA", bufs=3, space="PSUM") as psA,
            tc.tile_pool(name="psY", bufs=2, space="PSUM") as psY,
        ):
            # ---- persistent SBUF tensors ----
            xt_sb = big.tile([128, 8, T], MMDT)
            wq_sb = big.tile([128, 8, JW], MMDT)
            wk_sb = big.tile([128, 8, JW], MMDT)
            wv_sb = big.tile([128, 8, JW], MMDT)
            wo_sb = big.tile([128, 2, C], MMDT)
            qt_sb = big.tile([128, 2, T], MMDT)
            kt_sb = big.tile([128, 2, T], MMDT)
            yt_sb = big.tile([128, 2, T], MMDT)
            # V natural + 64 ones columns per head (denominator broadcast rows)
            v_sb = big.tile([128, 16, HPG, 128], MMDT)
            bq_sb = big.tile([128, 2], F32)
            bk_sb = big.tile([128, 2], F32)

            # FIFO per HWDGE ring: order loads by first use (wv/wo after xt)
            nc.sync.dma_start(out=wq_sb[:], in_=wq_ap)
            nc.sync.dma_start(out=wk_sb[:], in_=wk_ap)
            nc.sync.dma_start(out=bq_sb[:], in_=bq_ap)
            nc.sync.dma_start(out=bk_sb[:], in_=bk_ap)
            nc.vector.memset(v_sb[:, :, :, HD:128], 1.0)
            for cc in range(8):
                nc.sync.dma_start(out=xt_sb[:, cc, :], in_=xt_ap[:, cc, :])
            nc.sync.dma_start(out=wv_sb[:], in_=wv_ap)
            nc.sync.dma_start(out=wo_sb[:], in_=wo_ap)

            # ---- phase 1: QT, KT (transposed, j on partitions), V natural ----
            for jt in range(2):
                for tb in range(2):
                    ts = slice(tb * 1024, (tb + 1) * 1024)
                    pq = psA.tile([128, 1024], F32, tag="mm")
                    pk = psA.tile([128, 1024], F32, tag="mm")
                    for half in range(2):
                        hs = slice(half * 512, (half + 1) * 512)
                        xs = slice(tb * 1024 + half * 512, tb * 1024 + half * 512 + 512)
                        for cc in range(8):
                            nc.tensor.matmul(
                                pq[:, hs],
                                wq_sb[:, cc, jt * 128 : (jt + 1) * 128],
                                xt_sb[:, cc, xs],
                                start=(cc == 0),
                                stop=(cc == 7),
                            )
                        for cc in range(8):
                            nc.tensor.matmul(
                                pk[:, hs],
                                wk_sb[:, cc, jt * 128 : (jt + 1) * 128],
                                xt_sb[:, cc, xs],
                                start=(cc == 0),
                                stop=(cc == 7),
                            )
                    nc.vector.tensor_scalar_add(
                        out=qt_sb[:, jt, ts], in0=pq[:], scalar1=bq_sb[:, jt : jt + 1]
                    )
                    nc.vector.tensor_scalar_add(
                        out=kt_sb[:, jt, ts], in0=pk[:], scalar1=bk_sb[:, jt : jt + 1]
                    )

            for tg in range(8):  # V: two t-chunks of 128 per psum tile
                pv = psA.tile([128, 1024], F32, tag="mm")
                for half in range(2):
                    tt = tg * 2 + half
                    for cc in range(8):
                        nc.tensor.matmul(
                            pv[:, half * 512 : half * 512 + JW],
                            xt_sb[:, cc, tt * 128 : (tt + 1) * 128],
                            wv_sb[:, cc, :],
                            start=(cc == 0),
                            stop=(cc == 7),
                        )
                pv3 = pv[:].rearrange("p (half j) -> p half j", half=2)
                nc.vector.tensor_copy(
                    out=v_sb[:, tg * 2 : tg * 2 + 2, :, 0:HD],
                    in_=pv3[:, :, 0:JW].rearrange("p half (h d) -> p half h d", h=HPG),
                )

            # ---- phase 2+3: attention per (q-block, head), out-proj per q-block ----
            for qb in range(4):
                qs = slice(qb * 512, (qb + 1) * 512)
                for h in range(HPG):
                    jt, pb = h // 2, 64 * (h % 2)
                    yext = psY.tile([128, 512], F32, tag="yext")
                    for kg in range(8):  # pairs of tk chunks
                        st = psA.tile([128, 1024], F32, tag="mm")
                        for half in range(2):
                            kc = kg * 2 + half
                            nc.tensor.matmul(
                                st[:, half * 512 : (half + 1) * 512],
                                kt_sb[pb : pb + HD, jt, kc * 128 : (kc + 1) * 128],
                                qt_sb[pb : pb + HD, jt, qs],
                                start=True,
                                stop=True,
                            )
                        es = work.tile([128, 1024], MMDT, tag="es")
                        nc.scalar.activation(
                            out=es[:], in_=st[:], func=mybir.ActivationFunctionType.Exp
                        )
                        for half in range(2):
                            kc = kg * 2 + half
                            nc.tensor.matmul(
                                yext[:],
                                v_sb[:, kc, h, :],
                                es[:, half * 512 : (half + 1) * 512],
                                start=(kc == 0),
                                stop=(kc == 15),
                            )
                    r32 = nrm.tile([HD, 512], F32, tag="r32")
                    nc.vector.reciprocal(out=r32[:], in_=yext[HD:128, :])
                    nc.vector.tensor_mul(
                        out=yt_sb[pb : pb + HD, jt, qs], in0=r32[:], in1=yext[0:HD, :]
                    )
                # out projection deferred one q-block so it never waits on the
                # normalization chain of the q-block just computed
                if qb > 0:
                    _outproj(nc, psA, outp, yt_sb, wo_sb, out, qb - 1)
            _outproj(nc, psA, outp, yt_sb, wo_sb, out, 3)

    nc.finalize()
    return nc


def _get_nc():
    global _CACHED_NC
    if _CACHED_NC is None:
        _CACHED_NC = _build()
    return _CACHED_NC


def make_in_maps(x, Wq, bq, Wk, bk, Wv, Wo):
    """Per-core input dicts (host-side sharding + layout + fp16 cast)."""
    xts = [
        np.ascontiguousarray(np.asarray(x[b], np.float32).T).astype(NPDT)
        for b in range(B)
    ]
    wq_f = np.asarray(Wq, np.float32) / 8.0
    wk_f = np.asarray(Wk, np.float32)
    wv_f = np.asarray(Wv, np.float32)
    wo_f = np.asarray(Wo, np.float32)
    bq_f = np.asarray(bq, np.float32) / 8.0
    bk_f = np.asarray(bk, np.float32)
    in_maps = []
    for c in range(NCORES):
        b, g = c // GROUPS, c % GROUPS
        js = slice(g * JW, (g + 1) * JW)
        in_maps.append(
            {
                "xt": xts[b],
                "wq": np.ascontiguousarray(wq_f[:, js]).astype(NPDT),
                "wk": np.ascontiguousarray(wk_f[:, js]).astype(NPDT),
                "wv": np.ascontiguousarray(wv_f[:, js]).astype(NPDT),
                "wo": np.ascontiguousarray(wo_f[js, :]).astype(NPDT),
                "bq": np.ascontiguousarray(bq_f[js]),
                "bk": np.ascontiguousarray(bk_f[js]),
            }
        )
    return in_maps


def combine(results, bias_row):
    """Sum per-core head-group partials and add the host-side bias row."""
    out = np.zeros((B, T, C), np.float32)
    for c in range(NCORES):
        out[c // GROUPS] += results[c]["out"]
    out += bias_row
    return out


def kernel(x, Wq, bq, Wk, bk, Wv, bv, Wo, bo):
    nc = _get_nc()
    in_maps = make_in_maps(x, Wq, bq, Wk, bk, Wv, Wo)
    res = run_bass_kernel_spmd(nc, in_maps, core_ids=list(range(NCORES)))
    bias_row = (
        np.asarray(bv, np.float32) @ np.asarray(Wo, np.float32)
        + np.asarray(bo, np.float32)
    ).astype(np.float32)
    return combine(res.results, bias_row)


# revision 14
# speedup vs baseline: 1.0606x; 1.0606x over previous
"""Multi-head self-attention (B=2, T=2048, C=1024, H=16) on 8 NeuronCores.

Sharding: core c -> (batch b = c//4, head-group g = c%4); each core computes
4 heads' attention for one batch plus its slice of the QKV/out projections.
Per-core partial outputs (over head groups) are summed on the host.

Device-side layout is fully transposed (feature dim on partitions):
  xt [C, T] -> QT/KT [256, T] (j on partitions), V natural [T, 256],
  ST = K Qt (scores transposed, tk on partitions).
The stationary PV operand is V extended with 64 columns of ones, so the
yext accumulator's rows 64..127 all hold the softmax denominator — a free
hardware broadcast that lets normalization run entirely on VectorE
(reciprocal + multiply) without touching the PE or ScalarE.
Exp runs on ScalarE over 2-PSUM-bank tiles to amortize the ~352-cycle
per-instruction overhead. Matmul operands are fp16 (10-bit mantissa,
~6e-4 end-to-end rel err) with fp32 PSUM accumulation. bq/bk (and the
1/sqrt(hd) scale) are folded in on the host; bv/bo commute through softmax
(rows sum to 1) and are added on the host at the end.
"""

import numpy as np

import concourse.bacc as bacc
import concourse.mybir as mybir
import concourse.tile as tile
from concourse.bass_utils import run_bass_kernel_spmd

B, T, C, H = 2, 2048, 1024, 16
HD = C // H  # 64
NCORES = 8
GROUPS = 4  # head groups (one per core within a batch)
HPG = H // GROUPS  # heads per group = 4
JW = HPG * HD  # per-core projection slice width = 256

F32 = mybir.dt.float32
MMDT = mybir.dt.float16
NPDT = np.float16

_CACHED_NC = None


def _outproj(nc, psA, outp, yt_sb, wo_sb, out, qb):
    for tt in range(qb * 4, (qb + 1) * 4):
        po = psA.tile([128, 1024], F32, tag="mm", name="po")
        for mb in range(2):
            for jt in range(2):
                nc.tensor.matmul(
                    po[:, mb * 512 : (mb + 1) * 512],
                    yt_sb[:, jt, tt * 128 : (tt + 1) * 128],
                    wo_sb[:, jt, mb * 512 : (mb + 1) * 512],
                    start=(jt == 0),
                    stop=(jt == 1),
                )
        ob = outp.tile([128, 1024], F32, tag="ob", name="ob")
        nc.vector.tensor_copy(out=ob[:], in_=po[:])
        nc.sync.dma_start(out=out[tt * 128 : (tt + 1) * 128, :], in_=ob[:])


def _build():
    nc = bacc.Bacc("TRN2", target_bir_lowering=False, num_devices=NCORES)

    xt = nc.dram_tensor("xt", [C, T], MMDT, kind="ExternalInput")
    wq = nc.dram_tensor("wq", [C, JW], MMDT, kind="ExternalInput")
    wk = nc.dram_tensor("wk", [C, JW], MMDT, kind="ExternalInput")
    wv = nc.dram_tensor("wv", [C, JW], MMDT, kind="ExternalInput")
    wo = nc.dram_tensor("wo", [JW, C], MMDT, kind="ExternalInput")
    bq = nc.dram_tensor("bq", [JW], F32, kind="ExternalInput")
    bk = nc.dram_tensor("bk", [JW], F32, kind="ExternalInput")
    out = nc.dram_tensor("out", [T, C], F32, kind="ExternalOutput")

    xt_ap = xt[:, :].rearrange("(cc p) t -> p cc t", p=128)  # [128, 8, T]
    wq_ap = wq[:, :].rearrange("(cc p) j -> p cc j", p=128)  # [128, 8, 256]
    wk_ap = wk[:, :].rearrange("(cc p) j -> p cc j", p=128)
    wv_ap = wv[:, :].rearrange("(cc p) j -> p cc j", p=128)
    wo_ap = wo[:, :].rearrange("(jt p) m -> p jt m", p=128)  # [128, 2, C]
    bq_ap = bq[:].rearrange("(jt p) -> p jt", p=128)  # [128, 2]
    bk_ap = bk[:].rearrange("(jt p) -> p jt", p=128)

    with tile.TileContext(nc) as tc:
        with (
            tc.tile_pool(name="big", bufs=1) as big,
            tc.tile_pool(name="work", bufs=6) as work,
            tc.tile_pool(name="nrm", bufs=4) as nrm,
            tc.tile_pool(name="outp", bufs=3) as outp,
            tc.tile_pool(name="psA", bufs=3, space="PSUM") as psA,
            tc.tile_pool(name="psY", bufs=2, space="PSUM") as psY,
        ):
            # ---- persistent SBUF tensors ----
            xt_sb = big.tile([128, 8, T], MMDT)
            wq_sb = big.tile([128, 8, JW], MMDT)
            wk_sb = big.tile([128, 8, JW], MMDT)
            wv_sb = big.tile([128, 8, JW], MMDT)
            wo_sb = big.tile([128, 2, C], MMDT)
            qt_sb = big.tile([128, 2, T], MMDT)
            kt_sb = big.tile([128, 2, T], MMDT)
            yt_sb = big.tile([128, 2, T], MMDT)
            # V natural + 64 ones columns per head (denominator broadcast rows)
            v_sb = big.tile([128, 16, HPG, 128], MMDT)
            bq_sb = big.tile([128, 2], F32)
            bk_sb = big.tile([128, 2], F32)

            # FIFO per HWDGE ring: order loads by first use (wv/wo after xt)
            nc.sync.dma_start(out=wq_sb[:], in_=wq_ap)
            nc.sync.dma_start(out=wk_sb[:], in_=wk_ap)
            nc.sync.dma_start(out=bq_sb[:], in_=bq_ap)
            nc.sync.dma_start(out=bk_sb[:], in_=bk_ap)
            nc.vector.memset(v_sb[:, :, :, HD:128], 1.0)
            for cc in range(8):
                nc.sync.dma_start(out=xt_sb[:, cc, :], in_=xt_ap[:, cc, :])
            nc.sync.dma_start(out=wv_sb[:], in_=wv_ap)
            nc.sync.dma_start(out=wo_sb[:], in_=wo_ap)

            # ---- phase 1: QT, KT (transposed, j on partitions), V natural ----
            for jt in range(2):
                for tb in range(2):
                    ts = slice(tb * 1024, (tb + 1) * 1024)
                    pq = psA.tile([128, 1024], F32, tag="mm")
                    pk = psA.tile([128, 1024], F32, tag="mm")
                    for half in range(2):
                        hs = slice(half * 512, (half + 1) * 512)
                        xs = slice(tb * 1024 + half * 512, tb * 1024 + half * 512 + 512)
                        for cc in range(8):
                            nc.tensor.matmul(
                                pq[:, hs],
                                wq_sb[:, cc, jt * 128 : (jt + 1) * 128],
                                xt_sb[:, cc, xs],
                                start=(cc == 0),
                                stop=(cc == 7),
                            )
                        for cc in range(8):
                            nc.tensor.matmul(
                                pk[:, hs],
                                wk_sb[:, cc, jt * 128 : (jt + 1) * 128],
                                xt_sb[:, cc, xs],
                                start=(cc == 0),
                                stop=(cc == 7),
                            )
                    nc.vector.tensor_scalar_add(
                        out=qt_sb[:, jt, ts], in0=pq[:], scalar1=bq_sb[:, jt : jt + 1]
                    )
                    nc.vector.tensor_scalar_add(
                        out=kt_sb[:, jt, ts], in0=pk[:], scalar1=bk_sb[:, jt : jt + 1]
                    )

            for tg in range(8):  # V: two t-chunks of 128 per psum tile
                pv = psA.tile([128, 1024], F32, tag="mm")
                for half in range(2):
                    tt = tg * 2 + half
                    for cc in range(8):
                        nc.tensor.matmul(
                            pv[:, half * 512 : half * 512 + JW],
                            xt_sb[:, cc, tt * 128 : (tt + 1) * 128],
                            wv_sb[:, cc, :],
                            start=(cc == 0),
                            stop=(cc == 7),
                        )
                pv3 = pv[:].rearrange("p (half j) -> p half j", half=2)
                nc.vector.tensor_copy(
                    out=v_sb[:, tg * 2 : tg * 2 + 2, :, 0:HD],
                    in_=pv3[:, :, 0:JW].rearrange("p half (h d) -> p half h d", h=HPG),
                )

            # ---- phase 2+3: attention per (q-block, head), out-proj per q-block ----
            for qb in range(4):
                qs = slice(qb * 512, (qb + 1) * 512)
                for h in range(HPG):
                    jt, pb = h // 2, 64 * (h % 2)
                    yext = psY.tile([128, 512], F32, tag="yext")
                    for kg in range(8):  # pairs of tk chunks
                        st = psA.tile([128, 1024], F32, tag="mm")
                        for half in range(2):
                            kc = kg * 2 + half
                            nc.tensor.matmul(
                                st[:, half * 512 : (half + 1) * 512],
                                kt_sb[pb : pb + HD, jt, kc * 128 : (kc + 1) * 128],
                                qt_sb[pb : pb + HD, jt, qs],
                                start=True,
                                stop=True,
                            )
                        es = work.tile([128, 1024], MMDT, tag="es")
                        nc.scalar.activation(
                            out=es[:], in_=st[:], func=mybir.ActivationFunctionType.Exp
                        )
                        for half in range(2):
                            kc = kg * 2 + half
                            nc.tensor.matmul(
                                yext[:],
                                v_sb[:, kc, h, :],
                                es[:, half * 512 : (half + 1) * 512],
                                start=(kc == 0),
                                stop=(kc == 15),
                            )
                    # normalization chunked by 128 cols to cut latency to the
                    # out-projection that consumes yt right after
                    for ch in range(4):
                        cs = slice(ch * 128, (ch + 1) * 128)
                        ys = slice(qb * 512 + ch * 128, qb * 512 + (ch + 1) * 128)
                        r32 = nrm.tile([HD, 128], F32, tag="r32")
                        nc.vector.reciprocal(out=r32[:], in_=yext[HD:128, cs])
                        nc.vector.tensor_mul(
                            out=yt_sb[pb : pb + HD, jt, ys],
                            in0=r32[:],
                            in1=yext[0:HD, cs],
                        )
                _outproj(nc, psA, outp, yt_sb, wo_sb, out, qb)

    nc.finalize()
    return nc


def _get_nc():
    global _CACHED_NC
    if _CACHED_NC is None:
        _CACHED_NC = _build()
    return _CACHED_NC


def make_in_maps(x, Wq, bq, Wk, bk, Wv, Wo):
    """Per-core input dicts (host-side sharding + layout + fp16 cast)."""
    xts = [
        np.ascontiguousarray(np.asarray(x[b], np.float32).T).astype(NPDT)
        for b in range(B)
    ]
    wq_f = np.asarray(Wq, np.float32) / 8.0
    wk_f = np.asarray(Wk, np.float32)
    wv_f = np.asarray(Wv, np.float32)
    wo_f = np.asarray(Wo, np.float32)
    bq_f = np.asarray(bq, np.float32) / 8.0
    bk_f = np.asarray(bk, np.float32)
    in_maps = []
    for c in range(NCORES):
        b, g = c // GROUPS, c % GROUPS
        js = slice(g * JW, (g + 1) * JW)
        in_maps.append(
            {
                "xt": xts[b],
                "wq": np.ascontiguousarray(wq_f[:, js]).astype(NPDT),
                "wk": np.ascontiguousarray(wk_f[:, js]).astype(NPDT),
                "wv": np.ascontiguousarray(wv_f[:, js]).astype(NPDT),
                "wo": np.ascontiguousarray(wo_f[js, :]).astype(NPDT),
                "bq": np.ascontiguousarray(bq_f[js]),
                "bk": np.ascontiguousarray(bk_f[js]),
            }
        )
    return in_maps


def combine(results, bias_row):
    """Sum per-core head-group partials and add the host-side bias row."""
    out = np.zeros((B, T, C), np.float32)
    for c in range(NCORES):
        out[c // GROUPS] += results[c]["out"]
    out += bias_row
    return out


def kernel(x, Wq, bq, Wk, bk, Wv, bv, Wo, bo):
    nc = _get_nc()
    in_maps = make_in_maps(x, Wq, bq, Wk, bk, Wv, Wo)
    res = run_bass_kernel_spmd(nc, in_maps, core_ids=list(range(NCORES)))
    bias_row = (
        np.asarray(bv, np.float32) @ np.asarray(Wo, np.float32)
        + np.asarray(bo, np.float32)
    ).astype(np.float32)
    return combine(res.results, bias_row)
